# revision 9
# baseline (speedup 1.0000x reference)
"""Trainium2 Bass kernel for nn_ExpertizedLinear (MoE routing, 8 experts, top-2).

Strategy (expert-parallel, per the sharding hint):
  - The tiny router (0.4% of FLOPs) runs on host in fp32: normalize, logits,
    softmax, top-2, renormalized combine weights.
  - Dispatch = host-side all-to-all: for each expert e, gather its selected
    tokens, fold the combine weight into the activations (the expert map is
    linear, so c * ((x Wa) Wb) == ((c*x) Wa) Wb), cast to bf16, transpose to
    [D, C] so the contraction dim lands on SBUF partitions.
  - Core e computes Y_e = (X_e @ Wa_e) @ Wb_e with bf16 matmuls (fp32 PSUM
    accumulation). fp32 matmul on TRN2 PE costs 4 cycles/row vs 1 for bf16,
    and DMA is the bottleneck anyway, so bf16 I/O halves the critical path.
  - Combine = host-side scatter-add of the two expert outputs per token.
"""

import math
import os
import sys
from contextlib import ExitStack

import numpy as np

# The concourse stack must see the axon jax platform; a stray JAX_PLATFORMS=cpu
# would hide the NeuronCores from bass2jax.
if os.environ.get("JAX_PLATFORMS", None) == "cpu" and "jax" not in sys.modules:
    os.environ.pop("JAX_PLATFORMS")

for _p in ("/opt/trn_rl_repo",):
    if _p not in sys.path and os.path.isdir(_p):
        sys.path.insert(0, _p)

import ml_dtypes  # noqa: E402

import concourse.tile as tile  # noqa: E402
from concourse import bacc, mybir  # noqa: E402
from concourse.bass_utils import run_bass_kernel_spmd  # noqa: E402

BF16 = mybir.dt.bfloat16
NP_BF16 = ml_dtypes.bfloat16
F32 = mybir.dt.float32

N_EXPERTS = 8
D = 2048  # in features (contraction dim of mm1)
R = 128  # expert rank
O = 2048  # out features
KC = D // 128  # 16 contraction chunks for mm1
TB = 512  # token block (PSUM bank = 512 fp32)

_PROGRAM_CACHE: dict[int, object] = {}
LAST_RUN = {"exec_time_ns": None, "mean_exec_time_ns": None}


def _build_program(C: int):
    """One-expert program, run SPMD on all 8 cores with per-core data.

    Inputs : xT [D, C] bf16 (tokens transposed, combine weight pre-folded)
             wa [D, R] bf16, wb [R, O] bf16
    Output : y  [C, O] bf16
    """
    assert C % 128 == 0
    nc = bacc.Bacc("TRN2", target_bir_lowering=False, debug=False, num_devices=1)
    # wa is host-pre-swizzled to [128, KC*R] (partition-major) so its DMA
    # runs with 4KB contiguous lines instead of 256B ones.
    xT = nc.dram_tensor("xT", [D, C], BF16, kind="ExternalInput").ap()
    wa = nc.dram_tensor("wa", [128, KC * R], BF16, kind="ExternalInput").ap()
    wb = nc.dram_tensor("wb", [R, O], BF16, kind="ExternalInput").ap()
    y = nc.dram_tensor("y", [C, O], BF16, kind="ExternalOutput").ap()

    n_blk = math.ceil(C / TB)
    xTr = xT.rearrange("(kc p) t -> p kc t", p=128)

    with tile.TileContext(nc) as tc, ExitStack() as ctx:
        wpool = ctx.enter_context(tc.tile_pool(name="w", bufs=1))
        xpool = ctx.enter_context(tc.tile_pool(name="x", bufs=4))
        hpool = ctx.enter_context(tc.tile_pool(name="h", bufs=2))
        ypool = ctx.enter_context(tc.tile_pool(name="y", bufs=3))
        hps = ctx.enter_context(tc.tile_pool(name="hps", bufs=2, space="PSUM"))
        yps = ctx.enter_context(tc.tile_pool(name="yps", bufs=3, space="PSUM"))

        wa_sb = wpool.tile([128, KC, R], BF16)
        nc.sync.dma_start(wa_sb[:], wa.rearrange("p (kc r) -> p kc r", kc=KC))
        wb_sb = wpool.tile([128, O], BF16)
        nc.sync.dma_start(wb_sb[:], wb[:])

        for b in range(n_blk):
            t0 = b * TB
            tb = min(TB, C - t0)
            n_grp = math.ceil(tb / 128)

            xt = xpool.tile([128, KC, TB], BF16, tag="xt")
            # Split the block load along kc so the first mm1 of block 0 can
            # start after 1/4 of the block has landed.
            for q in range(2):
                nc.sync.dma_start(
                    xt[:, q * 8 : (q + 1) * 8, :tb],
                    xTr[:, q * 8 : (q + 1) * 8, t0 : t0 + tb],
                )

            # mm1: hT[r, t] += wa[d,r].T @ xT[d, t], accumulated over 16 d-chunks
            hp = hps.tile([128, TB], F32, tag="hp")
            for kc in range(KC):
                nc.tensor.matmul(
                    hp[:, :tb],
                    wa_sb[:, kc, :],
                    xt[:, kc, :tb],
                    start=(kc == 0),
                    stop=(kc == KC - 1),
                )
            hs = hpool.tile([128, TB], BF16, tag="hs")
            nc.any.tensor_copy(hs[:, :tb], hp[:, :tb])

            # mm2: y[t, o] = h[r, t].T @ wb[r, o], 128 tokens / 512 cols per MM
            ys = ypool.tile([128, 4, O], BF16, tag="ys")
            for g in range(n_grp):
                gt = min(128, tb - g * 128)
                lhs = hs[:, g * 128 : g * 128 + gt]
                for half in range(2):
                    yp = yps.tile([128, 1024], F32, tag="yp")
                    for j in range(2):
                        c0 = half * 1024 + j * 512
                        nc.tensor.matmul(
                            yp[:gt, j * 512 : (j + 1) * 512],
                            lhs,
                            wb_sb[:, c0 : c0 + 512],
                            start=True,
                            stop=True,
                        )
                    nc.any.tensor_copy(
                        ys[:gt, g, half * 1024 : (half + 1) * 1024], yp[:gt, :]
                    )
                nc.gpsimd.dma_start(
                    y[t0 + g * 128 : t0 + g * 128 + gt, :], ys[:gt, g, :]
                )

    nc.compile()
    return nc


def _get_program(C: int):
    if C not in _PROGRAM_CACHE:
        _PROGRAM_CACHE[C] = _build_program(C)
    return _PROGRAM_CACHE[C]


def _route(x: np.ndarray, router_w: np.ndarray):
    """fp32 host router matching the reference semantics."""
    norm = np.maximum(np.sqrt(np.einsum("td,td->t", x, x, dtype=np.float64)), 1e-12)
    logits = (x @ router_w) / norm[:, None].astype(np.float32)
    m = logits.max(-1, keepdims=True)
    p = np.exp(logits - m, dtype=np.float32)
    p /= p.sum(-1, keepdims=True)
    t_idx = np.arange(x.shape[0])
    e1 = p.argmax(-1)
    w1 = p[t_idx, e1]
    p2 = p.copy()
    p2[t_idx, e1] = -np.inf
    e2 = p2.argmax(-1)
    w2 = p[t_idx, e2]
    s = w1 + w2
    return e1, e2, (w1 / s).astype(np.float32), (w2 / s).astype(np.float32)


def kernel(hidden_states, router_w, Wa, Wb):
    B, S, _ = hidden_states.shape
    x = np.ascontiguousarray(
        np.asarray(hidden_states, dtype=np.float32).reshape(-1, D)
    )
    T = x.shape[0]
    router_w = np.asarray(router_w, dtype=np.float32)
    Wa = np.asarray(Wa, dtype=np.float32)
    Wb = np.asarray(Wb, dtype=np.float32)

    e1, e2, c1, c2 = _route(x, router_w)

    idxs, weights = [], []
    counts = np.zeros(N_EXPERTS, np.int64)
    for e in range(N_EXPERTS):
        m1 = e1 == e
        m2 = e2 == e
        idx = np.nonzero(m1 | m2)[0]
        c = np.where(m1[idx], c1[idx], c2[idx])
        idxs.append(idx)
        weights.append(c.astype(np.float32))
        counts[e] = idx.size

    C = max(128, int(math.ceil(counts.max() / 128.0)) * 128)
    nc = _get_program(C)

    in_maps = []
    for e in range(N_EXPERTS):
        idx, c = idxs[e], weights[e]
        xs = np.zeros((C, D), np.float32)
        xs[: idx.size] = x[idx] * c[:, None]
        xT = np.ascontiguousarray(xs.astype(NP_BF16).T)
        wa_sw = np.ascontiguousarray(
            Wa[e].reshape(KC, 128, R).transpose(1, 0, 2).reshape(128, KC * R)
        ).astype(NP_BF16)
        in_maps.append(
            {
                "xT": xT,
                "wa": wa_sw,
                "wb": Wb[e].astype(NP_BF16),
            }
        )

    trace = bool(int(os.environ.get("KERNEL_TRACE", "0")))
    for attempt in range(3):
        try:
            res = run_bass_kernel_spmd(
                nc,
                in_maps,
                list(range(N_EXPERTS)),
                trace=trace,
                trace_cores=list(range(N_EXPERTS)) if trace else None,
            )
            break
        except Exception:  # transient NRT_EXEC_UNIT_UNRECOVERABLE etc.
            if attempt == 2:
                raise
            import time as _time

            _time.sleep(2.0 * (attempt + 1))
    LAST_RUN["exec_time_ns"] = res.exec_time_ns
    LAST_RUN["mean_exec_time_ns"] = res.mean_exec_time_ns

    out = np.zeros((T, O), np.float32)
    for e in range(N_EXPERTS):
        idx = idxs[e]
        out[idx] += res.results[e]["y"][: idx.size].astype(np.float32)
    return out.reshape(B, S, O)
